# revision 41
# baseline (speedup 1.0000x reference)
"""Trainium2 Bass kernel for nn_CausalAttentionKVCache (B=2, T=2048, D=1024, 16 heads).

Sharding: 8 cores = 2 batch-halves x 4 head-groups (4 heads each).
Two compiled SPMD programs (one per batch-half, phase constants differ mod 3),
dispatched concurrently on jax devices [0:4] and [4:8].

The module's reshape y.view(3,B,T,hs,nh) scrambles tokens: flat row
v = (c*B*T + b*T + t)//3 of y=[x@W+b] in column block j=(c*B*T+b*T+t)%3 holds
token t of tensor c (q/k/v). With a host-side column permutation of W
(W2[:, j*1024+h*64+d] = W[:, j*1024+d*16+h]) each head's 64 features are
contiguous and all three tensors share the same weight/bias blocks (WQK/BQK):
q/k/v differ only in which x-row window feeds the projection and the
residue->column-block map.

All matmul operands are bf16 (PSUM stays f32; matmul cost is 1 cycle per
moving-dim column at any N). Q^T, K^T and V^T are all descrambled into token
order by strided PSUM evictions (DVE), so attention runs on contiguous
128-token chunks: the causal mask is a single 128-wide affine_select on the
diagonal chunk only. V^T is flipped to V[token, d] by PE identity-matmuls
(64 cycles each) with a ones-column appended for the softmax denominator.
S^T = K^T.T@Q^T (k on partitions, two 64-row PE tiles) -> exp on ScalarE
(scale=1/8 fused; scores ~N(0,1) so no max-subtraction) -> PV re-oriented
with P^T stationary: ctx[q,65] += P^T[k,128q-block].T @ V[k,65], 65 cycles
per 128x128 block (vs ~512 with V stationary). The epilogue ships the raw
ctx + denominator column to DRAM; the softmax division happens on the host
during the gather. Projections are split into 4 v-ranges with batched DMAs
(HWDGE charges ~625ns per transfer regardless of size) so the first
attention window starts ~7us in; the remaining splits stream as fillers
inside attention windows, each window self-filling work only its own late
chunks read (its K split, its V transposes). ctx PSUM is zeroed by DVE
memset because matmul start=True zeroes a whole 2KB PSUM bank.
"""
import sys
import os

sys.path.insert(0, "/opt/trn_rl_repo")

import numpy as np

import concourse.bass as bass
import concourse.bacc as bacc
import concourse.mybir as mybir
import concourse.tile as tile

B, T, D, NH, HS = 2, 2048, 1024, 16, 64
NV = 684          # v-rows per (c, batch-half) slice
NCHUNK = 16       # k/v chunks of 128 tokens
QW = 512          # q window
BF16 = mybir.dt.bfloat16
F32 = mybir.dt.float32
VS = [(0, 172), (172, 344), (344, 516), (516, 684)]  # v-range splits

_CACHE = {}


def _phase(B2):
    """Compile-time residue/offset constants for batch-half B2."""
    cst = {}
    for c in range(3):
        u0 = c * B * T + B2 * T
        vstart = u0 // 3
        rc_of_jj, r0_of_jj = {}, {}
        for rc in range(3):
            jj = (u0 + rc) % 3
            rc_of_jj[jj] = rc
            r0_of_jj[jj] = (u0 + rc - jj) // 3 - vstart
        cst[c] = dict(u0=u0, vstart=vstart, rc=rc_of_jj, r0=r0_of_jj)
    return cst


def _build_program(B2, repeat=1):
    cst = _phase(B2)
    nc = bacc.Bacc("TRN2", target_bir_lowering=False, debug=False, num_devices=4)

    xtq_d = nc.dram_tensor("XTQ", [D, 768], BF16, kind="ExternalInput")
    xtk_d = nc.dram_tensor("XTK", [D, 768], BF16, kind="ExternalInput")
    xtv_d = nc.dram_tensor("XTV", [D, 768], BF16, kind="ExternalInput")
    wqk_d = nc.dram_tensor("WQK", [D, 768], BF16, kind="ExternalInput")
    bqk_d = nc.dram_tensor("BQK", [128, 6], F32, kind="ExternalInput")
    id2_d = nc.dram_tensor("ID2", [128, 64], BF16, kind="ExternalInput")
    out_d = nc.dram_tensor("OUT", [2, 2, 4, 4, 128, 65], F32,
                           kind="ExternalOutput")

    xsrc = {0: xtq_d, 1: xtk_d, 2: xtv_d}

    with tile.TileContext(nc) as tc:
        with (
            tc.tile_pool(name="wpool", bufs=1) as wpool,
            tc.tile_pool(name="xpool", bufs=3) as xpool,
            tc.tile_pool(name="qkv", bufs=1) as qkvp,
            tc.tile_pool(name="ppool", bufs=8) as ppool,
            tc.tile_pool(name="opool", bufs=4) as opool,
        ):
            from contextlib import ExitStack
            wqk = wpool.tile([128, 8, 768], BF16)
            bqk = wpool.tile([128, 6], F32)
            id2 = wpool.tile([128, 64], BF16)

            for _rep in range(repeat):
                proj_ctx = ExitStack()
                psqk = proj_ctx.enter_context(
                    tc.tile_pool(name="psqk", bufs=int(os.environ.get("KPSQK", "4")), space="PSUM"))
                qt = qkvp.tile([128, 2, T], BF16, tag="qt")
                kt = qkvp.tile([128, 2, T], BF16, tag="kt")
                vt = qkvp.tile([128, 2, T], BF16, tag="vt")
                v_sb = qkvp.tile([128, NCHUNK, 4, 80], BF16, tag="v_sb")
                nc.vector.memset(v_sb[:, :, :, 64:65], 1.0)

                xts = {
                    si: xpool.tile([128, 8, 768], BF16, tag="xt",
                                   name=f"x{si}")
                    for si in range(3)
                }
                # Batched DMAs (HWDGE costs ~625ns/transfer regardless of
                # size): one DMA per (tensor, v-range) covering all 8
                # contraction chunks. Split-A columns of q, k, v land first
                # so the projection pipeline starts early.
                def xdma(si, lo, hi):
                    nc.sync.dma_start(
                        xts[si][:, :, lo:hi],
                        xsrc[si].rearrange("(c p) v -> p c v", p=128)
                        [:, :, lo:hi])

                lo_a, hi_a = VS[0]
                hi_b = VS[1][1]

                def wdma(i0, i1):
                    if _rep == 0:
                        nc.sync.dma_start(
                            wqk[:, i0:i1, :],
                            wqk_d.rearrange("(c p) f -> p c f", p=128)
                            [:, i0:i1, :])

                def xdma_ic(si, i0, i1, lo, hi):
                    nc.sync.dma_start(
                        xts[si][:, i0:i1, lo:hi],
                        xsrc[si].rearrange("(c p) v -> p c v", p=128)
                        [:, i0:i1, lo:hi])

                # fine-grained interleave so the first Q-A matmuls start
                # ~3.5us in instead of waiting for whole-tensor transfers
                wdma(0, 2)
                xdma_ic(0, 0, 4, lo_a, hi_a)
                wdma(2, 4)
                xdma_ic(0, 4, 8, lo_a, hi_a)
                if _rep == 0:
                    nc.sync.dma_start(bqk[:], bqk_d[:, :])
                    nc.sync.dma_start(id2[:], id2_d[:, :])
                wdma(4, 6)
                wdma(6, 8)
                xdma(1, lo_a, hi_a)
                xdma(2, lo_a, hi_a)
                for lo_r, hi_r in ((hi_a, hi_b), (hi_b, 768)):
                    for si in (0, 2, 1):
                        xdma(si, lo_r, hi_r)

                # ---- projection emitter (q/k/v unified) ----
                def emit_proj(si, fc, k, pool=None, tag="psqk"):
                    jj, sub = fc // 2, fc % 2
                    rc = cst[si]["rc"][jj]
                    r0 = cst[si]["r0"][jj]
                    nrc = 683 if rc < 2 else 682
                    lo, hi = VS[k]
                    n = hi - lo
                    ps = (pool or psqk).tile([128, 172], F32, tag=tag,
                                             name="psp")
                    for ic in range(8):
                        nc.tensor.matmul(
                            ps[:, 0:n],
                            wqk[:, ic, fc * 128:(fc + 1) * 128],
                            xts[si][:, ic, lo:hi],
                            start=(ic == 0),
                            stop=(ic == 7),
                        )
                    vv0 = max(lo, r0)
                    vv1 = min(hi, r0 + nrc)
                    if vv1 <= vv0:
                        return
                    t0 = 3 * (vv0 - r0) + rc
                    t1 = min(t0 + 3 * (vv1 - vv0), T)
                    dst = (qt, kt, vt)[si]
                    nc.vector.tensor_scalar_add(
                        dst[:, sub, t0:t1:3],
                        ps[:, vv0 - lo: vv1 - lo],
                        bqk[:, fc: fc + 1],
                    )

                def emit_vtr(m, pool=None, tag="psqk"):
                    # V chunk transpose on the PE: identity as the moving
                    # operand (64 cycles), DVE copy evicts to v_sb in bf16.
                    for h in range(4):
                        fg, hr2 = h // 2, h % 2
                        ps = (pool or psqk).tile([128, 64], F32, tag=tag,
                                                 name="trp")
                        nc.tensor.matmul(
                            ps[:],
                            vt[hr2 * 64:(hr2 + 1) * 64, fg,
                               128 * m:128 * (m + 1)],
                            id2[hr2 * 64:(hr2 + 1) * 64, :],
                            start=True,
                            stop=True,
                            tile_position=(hr2 * 64, 0),
                        )
                        nc.vector.tensor_copy(v_sb[:, m, h, 0:64], ps[:])

                # ---- attention emitters ----
                def emit_s_exp(hp, q0, m):
                    a = max(0, 128 * m - q0)
                    s_ps = pss.tile([128, 2 * QW], F32, tag="s", name="s_ps")
                    for hr in range(2):
                        pr = slice(hr * 64, hr * 64 + 64)
                        nc.tensor.matmul(
                            s_ps[:, hr * QW + a: (hr + 1) * QW],
                            kt[pr, hp, 128 * m: 128 * (m + 1)],
                            qt[pr, hp, q0 + a: q0 + QW],
                            start=True,
                            stop=True,
                            tile_position=(hr * 64, 0),
                        )
                    p_sb = ppool.tile([128, 2, QW], BF16, tag="p", name="p_sb")
                    s3 = s_ps[:].rearrange("p (h w) -> p h w", h=2)
                    nc.scalar.activation(
                        p_sb[:, :, a:QW],
                        s3[:, :, a:QW],
                        mybir.ActivationFunctionType.Exp,
                        scale=float(HS) ** -0.5,
                    )
                    if 128 * m >= q0:   # diagonal chunk: causal staircase
                        nc.gpsimd.affine_select(
                            out=p_sb[:, :, a:a + 128],
                            in_=p_sb[:, :, a:a + 128],
                            pattern=[[0, 2], [1, 128]],
                            compare_op=mybir.AluOpType.is_ge,
                            fill=0.0,
                            base=q0 + a - 128 * m,
                            channel_multiplier=-1,
                        )
                    return p_sb

                def emit_ctx_zero(ctx):
                    # matmul start=True zeroes the whole 2KB PSUM bank, so
                    # the 4 qb sub-regions cannot each carry their own
                    # start flag: zero the tile on the DVE and accumulate
                    # everything with start=False.
                    for hr in range(2):
                        nc.vector.memset(ctx[hr][:, :, :], 0.0)

                def emit_pv(hp, ctx, q0, m, p_sb):
                    a = max(0, 128 * m - q0)
                    for hr in range(2):
                        h_loc = 2 * hp + hr
                        for qb in range(a // 128, 4):
                            nc.tensor.matmul(
                                ctx[hr][:, qb, :],
                                p_sb[:, hr, qb * 128:(qb + 1) * 128],
                                v_sb[:, m, h_loc, 0:65],
                                start=False,
                                stop=(m == q0 // 128 + qb),
                                skip_group_check=True,
                            )

                def make_epilogue(hp, qi, ctx, last):
                    # ship un-normalized ctx + denominator straight from
                    # PSUM; the softmax division happens on the host during
                    # the gather.
                    def epi():
                        o_sb = opool.tile([128, 2, 4, 65], F32, tag="o",
                                          name="o_sb")
                        for hr in range(2):
                            nc.vector.tensor_copy(
                                o_sb[:, hr, :, :], ctx[hr][:, :, :])
                            nc.sync.dma_start(
                                out_d[hp, hr, qi].rearrange(
                                    "qb p e -> p qb e"),
                                o_sb[:, hr, :, :],
                            )
                        if not last:
                            emit_ctx_zero(ctx)  # single-buffered ctx
                    return epi

                # ---- emission schedule ----
                # lead-in: split-A projections for hp0 attention + V, then
                # the first four V chunks transposed
                for fc in (0, 2, 4):
                    emit_proj(0, fc, 0)
                for fc in (0, 2, 4):
                    emit_proj(1, fc, 0)
                if not os.environ.get("KVAFILL"):
                    for fc in range(6):
                        emit_proj(2, fc, 0)
                    for m in range(4):
                        emit_vtr(m)
                proj_ctx.close()
                attn_ctx = ExitStack()
                pss = attn_ctx.enter_context(
                    tc.tile_pool(name="pss", bufs=2, space="PSUM"))
                psctx = attn_ctx.enter_context(
                    tc.tile_pool(name="psctx", bufs=1, space="PSUM"))
                psf = attn_ctx.enter_context(
                    tc.tile_pool(name="psf", bufs=2, space="PSUM"))

                def fp(si, fc, k):
                    return lambda: emit_proj(si, fc, k, pool=psf, tag="f")

                def ftr(m):
                    return lambda: emit_vtr(m, pool=psf, tag="f")

                # Each window self-fills work only its own LATE chunks (or a
                # later window) read: its K split (chunks m >= 4k), its V
                # transposes (PV lags S by DEPTH), next splits of Q/V.
                if os.environ.get("KILV"):
                    worder = [(0, 0), (0, 1), (1, 0), (1, 1),
                              (0, 2), (1, 2), (0, 3), (1, 3)]
                    fillers = {
                        (0, 0): [fp(0, fc, 1) for fc in (0, 2, 4)]
                                + [fp(2, fc, 1) for fc in range(6)],
                        (0, 1): [fp(1, fc, 1) for fc in (0, 2, 4)]
                                + [ftr(m) for m in (4, 5, 6, 7)]
                                + [fp(si, fc, 0) for si in (0, 1)
                                   for fc in (1, 3, 5)],
                        (1, 0): [fp(0, fc, 1) for fc in (1, 3, 5)]
                                + [fp(0, fc, 2) for fc in (0, 2, 4)],
                        (1, 1): [fp(1, fc, 1) for fc in (1, 3, 5)]
                                + [fp(2, fc, 2) for fc in range(6)]
                                + [ftr(m) for m in (8, 9, 10, 11)],
                        (0, 2): [fp(1, fc, 2) for fc in (0, 2, 4)]
                                + [fp(0, fc, 3) for fc in (0, 2, 4)]
                                + [fp(0, fc, 2) for fc in (1, 3, 5)],
                        (1, 2): [fp(1, fc, 2) for fc in (1, 3, 5)]
                                + [fp(2, fc, 3) for fc in range(6)]
                                + [ftr(m) for m in (12, 13, 14, 15)],
                        (0, 3): [fp(1, fc, 3) for fc in (0, 2, 4)]
                                + [fp(0, fc, 3) for fc in (1, 3, 5)],
                        (1, 3): [fp(1, fc, 3) for fc in (1, 3, 5)],
                    }
                else:
                    worder = [(0, 0), (0, 1), (0, 2), (0, 3),
                              (1, 0), (1, 1), (1, 2), (1, 3)]
                    fillers = {
                        (0, 0): ([fp(2, fc, 0) for fc in range(6)]
                                 + [ftr(m) for m in range(4)]
                                 if os.environ.get("KVAFILL") else [])
                                + [fp(0, fc, 1) for fc in (0, 2, 4)]
                                + [fp(2, fc, 1) for fc in range(6)],
                        (0, 1): [fp(1, fc, 1) for fc in (0, 2, 4)]
                                + [ftr(m) for m in (4, 5, 6, 7)]
                                + [fp(0, fc, 2) for fc in (0, 2, 4)]
                                + [fp(2, fc, 2) for fc in range(6)],
                        (0, 2): [fp(1, fc, 2) for fc in (0, 2, 4)]
                                + [ftr(m) for m in (8, 9, 10, 11)]
                                + [fp(0, fc, 3) for fc in (0, 2, 4)]
                                + [fp(2, fc, 3) for fc in range(6)],
                        (0, 3): [fp(1, fc, 3) for fc in (0, 2, 4)]
                                + [ftr(m) for m in (12, 13, 14, 15)]
                                + [fp(si, fc, 0) for si in (0, 1)
                                   for fc in (1, 3, 5)],
                        (1, 0): [fp(0, fc, 1) for fc in (1, 3, 5)],
                        (1, 1): [fp(1, fc, 1) for fc in (1, 3, 5)]
                                + [fp(0, fc, 2) for fc in (1, 3, 5)],
                        (1, 2): [fp(1, fc, 2) for fc in (1, 3, 5)]
                                + [fp(0, fc, 3) for fc in (1, 3, 5)],
                        (1, 3): [fp(1, fc, 3) for fc in (1, 3, 5)],
                    }

                DEPTH = int(os.environ.get('KDEPTH', '6'))
                deferred_epi = None
                for wi, (hp, qi) in enumerate(worder):
                    if True:
                        q0 = qi * QW
                        nm = q0 // 128 + 4
                        fill = list(fillers.get((hp, qi), []))
                        ctx = [
                            psctx.tile([128, 4, 65], F32, tag=f"ctx{hr}",
                                       name=f"ctx{hr}")
                            for hr in range(2)
                        ]
                        if wi == 0:
                            emit_ctx_zero(ctx)
                        pend = []
                        for m in range(nm):
                            pend.append((m, emit_s_exp(hp, q0, m)))
                            if m == 2 and deferred_epi is not None:
                                deferred_epi()
                                deferred_epi = None
                            if fill:
                                fill.pop(0)()
                            if len(pend) > DEPTH:
                                m0, p0 = pend.pop(0)
                                emit_pv(hp, ctx, q0, m0, p0)
                        if deferred_epi is not None:
                            deferred_epi()
                            deferred_epi = None
                        while fill:
                            fill.pop(0)()
                        is_last = wi == len(worder) - 1
                        for m0, p0 in pend:
                            emit_pv(hp, ctx, q0, m0, p0)
                        deferred_epi = make_epilogue(hp, qi, ctx, is_last)
                if deferred_epi is not None:
                    deferred_epi()
                attn_ctx.close()

    nc.compile()
    return nc



# ---------------------------------------------------------------------------
# host-side data prep
# ---------------------------------------------------------------------------

def _perm_cols():
    perm = np.empty(3 * D, dtype=np.int64)
    for j in range(3):
        for h in range(NH):
            for d in range(HS):
                perm[j * D + h * HS + d] = j * D + d * NH + h
    return perm


def _host_dt():
    import ml_dtypes
    return ml_dtypes.bfloat16


def _core_inputs(xT, W2, b2, B2, HG):
    """xT/W2 already in the matmul host dtype; b2 f32."""
    bf16 = _host_dt()
    cst = _phase(B2)

    def xt_slice(c):
        vs = cst[c]["vstart"]
        sl = np.zeros((D, 768), dtype=bf16)
        lo, hi = max(0, vs), min(B * T, vs + 768)
        sl[:, lo - vs: hi - vs] = xT[:, lo:hi]
        return sl

    WQK = np.empty((D, 768), dtype=bf16)
    BQKf = np.empty(768, dtype=np.float32)
    for jj in range(3):
        src = jj * D + HG * 256
        WQK[:, jj * 256:(jj + 1) * 256] = W2[:, src:src + 256]
        BQKf[jj * 256:(jj + 1) * 256] = b2[src:src + 256]
    BQK = BQKf.reshape(6, 128).T.copy()  # [128, 6]: col fc, partition p

    return {
        "XTQ": xt_slice(0),
        "XTK": xt_slice(1),
        "XTV": xt_slice(2),
        "WQK": WQK,
        "BQK": np.ascontiguousarray(BQK),
        "ID2": np.vstack([np.eye(64)] * 2).astype(bf16),
    }


# ---------------------------------------------------------------------------
# concurrent two-program dispatch (4+4 cores)
# ---------------------------------------------------------------------------

def _sharded_fn(nc, dev_lo, dev_hi):
    import jax
    from jax.sharding import Mesh, PartitionSpec
    from jax.experimental.shard_map import shard_map
    from concourse import bass2jax
    from concourse.bass2jax import _bass_exec_p, install_neuronx_cc_hook

    install_neuronx_cc_hook()
    n_cores = dev_hi - dev_lo

    in_names, out_names, out_avals, zero_shapes = [], [], [], []
    partition_name = (
        nc.partition_id_tensor.name if nc.partition_id_tensor else None
    )
    for alloc in nc.m.functions[0].allocations:
        if not isinstance(alloc, mybir.MemoryLocationSet):
            continue
        name = alloc.memorylocations[0].name
        if alloc.kind == "ExternalInput":
            if name != partition_name:
                in_names.append(name)
        elif alloc.kind == "ExternalOutput":
            np_dt = mybir.dt.np(alloc.dtype)
            out_avals.append(
                jax.core.ShapedArray(tuple(alloc.tensor_shape), np_dt)
            )
            out_names.append(name)
            zero_shapes.append((tuple(alloc.tensor_shape), np_dt))
    n_params = len(in_names)
    all_in_names = list(in_names) + list(out_names)
    if partition_name is not None:
        all_in_names.append(partition_name)

    donate = tuple(range(n_params, n_params + len(out_names)))

    def _body(*args):
        operands = list(args)
        if partition_name is not None:
            operands.append(bass2jax.partition_id_tensor())
        outs = _bass_exec_p.bind(
            *operands,
            out_avals=tuple(out_avals),
            in_names=tuple(all_in_names),
            out_names=tuple(out_names),
            lowering_input_output_aliases=(),
            sim_require_finite=True,
            sim_require_nnan=True,
            nc=nc,
        )
        return tuple(outs)

    devices = jax.devices()[dev_lo:dev_hi]
    mesh = Mesh(np.asarray(devices), ("core",))
    in_specs = (PartitionSpec("core"),) * (n_params + len(out_names))
    out_specs = (PartitionSpec("core"),) * len(out_names)
    fn = jax.jit(
        shard_map(_body, mesh=mesh, in_specs=in_specs, out_specs=out_specs,
                  check_rep=False),
        donate_argnums=donate,
        keep_unused=True,
    )
    return fn, in_names, out_names, out_avals, zero_shapes, n_cores


def _concat_inputs(in_maps, in_names):
    return [
        np.concatenate([np.asarray(m[name]) for m in in_maps], axis=0)
        for name in in_names
    ]


def kernel(x, W_qkv, b_qkv):
    bf16 = _host_dt()
    x = np.asarray(x, dtype=np.float32)
    W_qkv = np.asarray(W_qkv, dtype=np.float32)
    b_qkv = np.asarray(b_qkv, dtype=np.float32)

    if "progs" not in _CACHE:
        _CACHE["progs"] = {
            B2: _build_program(B2, repeat=int(os.environ.get("KREPEAT", "1")))
            for B2 in range(2)
        }
        _CACHE["fns"] = {
            0: _sharded_fn(_CACHE["progs"][0], 0, 4),
            1: _sharded_fn(_CACHE["progs"][1], 4, 8),
        }

    perm = _perm_cols()
    W2 = W_qkv[:, perm].astype(bf16)
    b2 = b_qkv[perm]
    xT = np.ascontiguousarray(x.reshape(B * T, D).T).astype(bf16)

    results = {}
    pending = []
    for B2 in range(2):
        fn, in_names, out_names, out_avals, zero_shapes, n_cores = _CACHE["fns"][B2]
        in_maps = [_core_inputs(xT, W2, b2, B2, HG) for HG in range(4)]
        concat_in = _concat_inputs(in_maps, in_names)
        concat_zeros = [
            np.zeros((n_cores * s[0], *s[1:]), d) for (s, d) in zero_shapes
        ]
        out_arrs = fn(*concat_in, *concat_zeros)  # async dispatch
        pending.append((B2, out_names, out_avals, n_cores, out_arrs))

    out_full = np.zeros((B, T, D), dtype=np.float32)
    for B2, out_names, out_avals, n_cores, out_arrs in pending:
        per_core = np.asarray(out_arrs[0]).reshape(
            n_cores, 2, 2, 4, 4, 128, 65)
        for HG in range(4):
            arr = per_core[HG]                      # [hp, hr, qi, qb, p, 65]
            ctxv = arr[..., 0:64]
            den = arr[..., 64:65]
            o = ctxv / den                          # [hp, hr, qi, qb, p, 64]
            # token t = qi*512 + qb*128 + p; head col = (2*hp+hr)*64 + d
            o = o.transpose(2, 3, 4, 0, 1, 5).reshape(T, 256)
            out_full[B2, :, HG * 256:(HG + 1) * 256] = o
    return out_full
